# revision 22
# baseline (speedup 1.0000x reference)
"""Trainium2 Bass kernel for nn_Encoder_53274774340258.

One transformer encoder block (pre-norm), B=4 P=2048 D=768 H=12, with the
quirk that softmax normalizes over the HEAD axis (dim=1 of (B,H,P,P)).

Sharding: 8 cores = 4 batches x 2 query-halves. Each core computes K/V for
its whole batch (2048 tokens) and Q/attention/MLP for its 1024 queries.
No collectives. The host permutes tokens so each core's queries are always
columns 0:1024 of the shipped tensor (key order is irrelevant to attention),
keeping the compiled graph identical across cores (SPMD).

Layout: activations are kept transposed (feature-on-partition, token-on-free)
end to end; the host pre-transposes x and un-transposes the output. LN affine
params and the attention scale are folded into the weights on the host; LN
mean/var are computed on-device with ones-vector matmuls.
"""

import os
from contextlib import ExitStack

import ml_dtypes
import numpy as np

import concourse.bass as bass
import concourse.bacc as bacc
import concourse.mybir as mybir
import concourse.tile as tile
from concourse.bass_utils import run_bass_kernel_spmd
from concourse.tile import add_dep_helper

BF16 = mybir.dt.bfloat16
F32 = mybir.dt.float32
AF = mybir.ActivationFunctionType
ALU = mybir.AluOpType

B, P, D, H = 4, 2048, 768, 12
HD = D // H          # 64
DQKV = 3 * D         # 2304
DFF = 4 * D          # 3072
EPS = 1e-5
CI = D // 128        # 6 c-tiles
TK = P               # keys per core (full batch)
TQ = P // 2          # queries per core
NQG = 4              # query groups
QGS = TQ // NQG      # 256
NKB = TK // 128      # 16 key blocks
NCORES = 8

_CACHE = {}
_LAST_RESULTS = None


def _bcast_mid(ap2d, n):
    """View a (P, F) AP as (P, n, F) with a 0-step broadcast middle dim."""
    return bass.AP(
        tensor=ap2d.tensor,
        offset=ap2d.offset,
        ap=[ap2d.ap[0], [0, n], ap2d.ap[1]],
    )


def build_nc(has_cqkv, has_c1, has_c2):
    nc = bacc.Bacc()

    xt = nc.dram_tensor("xt_bf", [D, TK], BF16, kind="ExternalInput")
    xq = nc.dram_tensor("xq_f32", [D, TQ], F32, kind="ExternalInput")
    wqkv_d = nc.dram_tensor("w_qkv", [D, DQKV], BF16, kind="ExternalInput")
    wfc1_d = nc.dram_tensor("w_fc1", [D, DFF], BF16, kind="ExternalInput")
    wfc2_d = nc.dram_tensor("w_fc2", [DFF, D], BF16, kind="ExternalInput")
    cqkv_d = nc.dram_tensor("c_qkv", [DQKV, 1], F32, kind="ExternalInput")
    c1_d = nc.dram_tensor("c_fc1", [DFF, 1], F32, kind="ExternalInput")
    c2_d = nc.dram_tensor("c_fc2", [D, 1], F32, kind="ExternalInput")
    out_t = nc.dram_tensor("out_t", [D, TQ], F32, kind="ExternalOutput")

    with tile.TileContext(nc) as tc:
        _build(tc, nc, xt, xq, wqkv_d, wfc1_d, wfc2_d, cqkv_d, c1_d, c2_d,
               out_t, has_cqkv, has_c1, has_c2)
    nc.compile()
    return nc


def _build(tc, nc, xt, xq, wqkv_d, wfc1_d, wfc2_d, cqkv_d, c1_d, c2_d, out_t,
           has_cqkv, has_c1, has_c2):
    top = ExitStack()
    with top:
        pconst = top.enter_context(tc.tile_pool(name="pconst", bufs=1))
        ones_col = pconst.tile([128, 1], BF16)
        nc.vector.memset(ones_col, 1.0)
        ones_row = pconst.tile([1, 128], BF16)
        nc.vector.memset(ones_row, 1.0)
        eps_tile = pconst.tile([1, 1], F32)
        nc.vector.memset(eps_tile, EPS)

        cqkv_sb = c1_sb = c2_sb = None
        if has_cqkv:
            cqkv_sb = pconst.tile([128, DQKV // 128], F32)
            nc.sync.dma_start(
                out=cqkv_sb,
                in_=cqkv_d.rearrange("(j p) one -> p (j one)", p=128))
        if has_c1:
            c1_sb = pconst.tile([128, DFF // 128], F32)
            nc.sync.dma_start(
                out=c1_sb, in_=c1_d.rearrange("(j p) one -> p (j one)", p=128))
        if has_c2:
            c2_sb = pconst.tile([128, D // 128], F32)
            nc.sync.dma_start(
                out=c2_sb, in_=c2_d.rearrange("(j p) one -> p (j one)", p=128))

        # small PSUM pool shared by LN stats + broadcast matmuls (2 banks)
        p_small_ps = top.enter_context(
            tc.tile_pool(name="p_small_ps", bufs=2, space="PSUM"))
        p_srow = top.enter_context(tc.tile_pool(name="p_srow", bufs=1))

        def ln_stats_and_bcast(src_aps, n):
            """src_aps: CI (128, n) bf16 APs; returns (sb, mb): (128, n) bf16
            broadcast tiles for s = rsqrt(var+eps) and m = -mu*s."""
            sq = []
            for ci in range(CI):
                sqt = p_srow.tile([128, n], BF16, tag="sq", bufs=3,
                                  name=f"sq{ci}")
                nc.gpsimd.tensor_mul(out=sqt, in0=src_aps[ci],
                                     in1=src_aps[ci])
                sq.append(sqt)
            psx = p_small_ps.tile([1, n], F32, tag="smallps")
            psq = p_small_ps.tile([1, n], F32, tag="smallps")
            for ci in range(CI):
                nc.tensor.matmul(out=psx, lhsT=ones_col, rhs=src_aps[ci],
                                 start=(ci == 0), stop=(ci == CI - 1))
            for ci in range(CI):
                nc.tensor.matmul(out=psq, lhsT=ones_col, rhs=sq[ci],
                                 start=(ci == 0), stop=(ci == CI - 1))
            mu = p_srow.tile([1, n], F32, tag="mu")
            nc.scalar.activation(out=mu, in_=psx, func=AF.Identity,
                                 scale=1.0 / D)
            ex2 = p_srow.tile([1, n], F32, tag="ex2")
            nc.scalar.activation(out=ex2, in_=psq, func=AF.Identity,
                                 scale=1.0 / D)
            var = p_srow.tile([1, n], F32, tag="var")
            nc.vector.tensor_mul(out=var, in0=mu, in1=mu)
            nc.vector.tensor_sub(out=var, in0=ex2, in1=var)
            # rsqrt(var+eps) = exp(-0.5*ln(var+eps)); Ln/Exp share a table set
            lnv = p_srow.tile([1, n], F32, tag="lnv")
            nc.scalar.activation(out=lnv, in_=var, func=AF.Ln, bias=eps_tile)
            s_bf = p_srow.tile([1, n], BF16, tag="sbf")
            nc.scalar.activation(out=s_bf, in_=lnv, func=AF.Exp, scale=-0.5)
            m_bf = p_srow.tile([1, n], BF16, tag="mbf")
            nc.vector.scalar_tensor_tensor(
                out=m_bf, in0=mu, scalar=-1.0, in1=s_bf,
                op0=ALU.mult, op1=ALU.mult)
            psb = p_small_ps.tile([128, n], F32, tag="smallps")
            nc.tensor.matmul(out=psb, lhsT=ones_row, rhs=s_bf,
                             start=True, stop=True)
            sb = p_srow.tile([128, n], BF16, tag="sb", bufs=2)
            nc.scalar.activation(out=sb, in_=psb, func=AF.Copy)
            psb2 = p_small_ps.tile([128, n], F32, tag="smallps")
            nc.tensor.matmul(out=psb2, lhsT=ones_row, rhs=m_bf,
                             start=True, stop=True)
            mb = p_srow.tile([128, n], BF16, tag="mb", bufs=2)
            nc.scalar.activation(out=mb, in_=psb2, func=AF.Copy)
            return sb, mb

        # long-lived output pools of the QKV phase
        p_qt = top.enter_context(tc.tile_pool(name="p_qt", bufs=1))
        p_kt = top.enter_context(tc.tile_pool(name="p_kt", bufs=1))
        p_vn = top.enter_context(tc.tile_pool(name="p_vn", bufs=1))
        qt_tiles = [p_qt.tile([128, TQ], BF16, tag=f"qt{j}", name=f"qt{j}")
                    for j in range(CI)]
        kt_tiles = [p_kt.tile([128, TK], BF16, tag=f"kt{j}", name=f"kt{j}")
                    for j in range(CI)]
        vn_tiles = [p_vn.tile([128, D], BF16, tag=f"vn{t}", name=f"vn{t}")
                    for t in range(NKB)]

        # ---------------- Phases A-C: LN1 -> h -> QKV ----------------
        with tc.tile_pool(name="p_h", bufs=1) as p_h, \
             tc.tile_pool(name="p_x", bufs=1) as p_x, \
             tc.tile_pool(name="p_wqkv", bufs=1) as p_wqkv, \
             tc.tile_pool(name="p_cps", bufs=4, space="PSUM") as p_cps:
            xt_tiles, h_tiles, wq_tiles = [], [], []
            for ci in range(CI):
                xtt = p_x.tile([128, TK], BF16, tag=f"xt{ci}", name=f"xt{ci}")
                nc.sync.dma_start(out=xtt, in_=xt[ci * 128:(ci + 1) * 128, :])
                xt_tiles.append(xtt)
                ht = p_h.tile([128, TK], BF16, tag=f"h{ci}", name=f"h{ci}")
                h_tiles.append(ht)
                wt = p_wqkv.tile([128, DQKV], BF16, tag=f"wqkv{ci}",
                                 name=f"wqkv{ci}")
                nc.sync.dma_start(out=wt,
                                  in_=wqkv_d[ci * 128:(ci + 1) * 128, :])
                wq_tiles.append(wt)

            for tg in range(TK // 512):
                cs = slice(tg * 512, (tg + 1) * 512)
                sb, mb = ln_stats_and_bcast(
                    [t[:, cs] for t in xt_tiles], 512)
                for ci in range(CI):
                    nc.gpsimd.tensor_mul(out=h_tiles[ci][:, cs],
                                         in0=xt_tiles[ci][:, cs], in1=sb)
                    nc.vector.tensor_add(out=h_tiles[ci][:, cs],
                                         in0=h_tiles[ci][:, cs], in1=mb)

            def proj_T(j0, dst, cols0, ncols, bias_col):
                ps = p_cps.tile([128, 512], F32, tag="cps", name="cps")
                for ci in range(CI):
                    nc.tensor.matmul(
                        out=ps[:, :ncols],
                        lhsT=wq_tiles[ci][:, j0:j0 + 128],
                        rhs=h_tiles[ci][:, cols0:cols0 + ncols],
                        start=(ci == 0), stop=(ci == CI - 1))
                if has_cqkv:
                    nc.scalar.activation(
                        out=dst[:, cols0:cols0 + ncols], in_=ps[:, :ncols],
                        func=AF.Identity,
                        bias=cqkv_sb[:, bias_col:bias_col + 1])
                else:
                    nc.scalar.activation(
                        out=dst[:, cols0:cols0 + ncols], in_=ps[:, :ncols],
                        func=AF.Copy)

            for jt in range(CI):            # Q^T: local queries only
                for tt in range(TQ // 512):
                    proj_T(jt * 128, qt_tiles[jt], tt * 512, 512, jt)
            for jt in range(CI):            # K^T: all tokens
                for tg in range(TK // 512):
                    proj_T(D + jt * 128, kt_tiles[jt], tg * 512, 512, CI + jt)
            # V natural: (token, dim) via lhsT = h block
            for tb in range(NKB):
                for (n0, nw) in ((0, 512), (512, 256)):
                    ps = p_cps.tile([128, 512], F32, tag="cps", name="cps")
                    for ci in range(CI):
                        nc.tensor.matmul(
                            out=ps[:, :nw],
                            lhsT=h_tiles[ci][:, tb * 128:(tb + 1) * 128],
                            rhs=wq_tiles[ci][:, 2 * D + n0:2 * D + n0 + nw],
                            start=(ci == 0), stop=(ci == CI - 1))
                    nc.vector.tensor_copy(
                        out=vn_tiles[tb][:, n0:n0 + nw], in_=ps[:, :nw])

        # ---------------- Phase D/E pools ----------------
        p_sps = top.enter_context(
            tc.tile_pool(name="p_sps", bufs=1, space="PSUM"))
        p_avps = top.enter_context(
            tc.tile_pool(name="p_avps", bufs=3, space="PSUM"))
        p_e = top.enter_context(tc.tile_pool(name="p_e", bufs=3))
        p_z = top.enter_context(tc.tile_pool(name="p_z", bufs=2))
        p_zr = top.enter_context(tc.tile_pool(name="p_zr", bufs=3))
        p_x2 = top.enter_context(tc.tile_pool(name="p_x2", bufs=1))
        p_xq = top.enter_context(tc.tile_pool(name="p_xq", bufs=2))
        p_wf1 = top.enter_context(tc.tile_pool(name="p_wf1", bufs=1))
        p_h2 = top.enter_context(tc.tile_pool(name="p_h2", bufs=2))
        p_g1 = top.enter_context(tc.tile_pool(name="p_g1", bufs=1))
        p_wf2 = top.enter_context(tc.tile_pool(name="p_wf2", bufs=6))
        p_y = top.enter_context(tc.tile_pool(name="p_y", bufs=3))
        p_x2b = top.enter_context(tc.tile_pool(name="p_x2b", bufs=1))

        x2_tiles = [p_x2.tile([128, TQ], F32, tag=f"x2_{j}", name=f"x2_{j}")
                    for j in range(CI)]
        wf1_tiles = []
        for ci in range(CI):
            wt = p_wf1.tile([128, DFF], BF16, tag=f"wfc1{ci}",
                            name=f"wfc1{ci}")
            nc.sync.dma_start(out=wt, in_=wfc1_d[ci * 128:(ci + 1) * 128, :])
            wf1_tiles.append(wt)

        def attention_qg(qg):
            q0 = qg * QGS
            av = [p_avps.tile([128, 512], F32, tag="av", name=f"av{qg}_{g}")
                  for g in range(3)]
            av_first = [[None, None] for _ in range(3)]
            av_last = [[None, None] for _ in range(3)]
            for kb in range(NKB):
                ek = p_e.tile([128, H, QGS], BF16, tag="ek",
                              name=f"ek{qg}_{kb}")
                # Concurrent row-group MMs must NOT share a PSUM bank
                # (HW fault) -> map paired heads to different banks:
                # slot order in sp/ek is [h0, h2, h1, h3] of each quad.
                SLOT = (0, 2, 1, 3)
                for pg in range(3):       # 4 heads per scores tile
                    sp = p_sps.tile([128, 4 * QGS], F32, tag="sp",
                                    name=f"sp{qg}_{kb}_{pg}")
                    mms = []
                    for hh in range(4):
                        hd = pg * 4 + hh
                        jt, pr = hd // 2, hd % 2
                        sl = SLOT[hh]
                        mm = nc.tensor.matmul(
                            out=sp[:, sl * QGS:(sl + 1) * QGS],
                            lhsT=kt_tiles[jt][pr * 64:(pr + 1) * 64,
                                              kb * 128:(kb + 1) * 128],
                            rhs=qt_tiles[jt][pr * 64:(pr + 1) * 64,
                                             q0:q0 + QGS],
                            start=(hh in (0, 1)), stop=(hh in (2, 3)),
                            tile_position=(pr * 64, 0))
                        mms.append(mm)
                    # each bank's stop MM must run after its start MM
                    add_dep_helper(mms[2].ins, mms[0].ins, reason="psum grp")
                    add_dep_helper(mms[3].ins, mms[1].ins, reason="psum grp")
                    nc.scalar.activation(
                        out=ek[:, pg * 4:(pg + 1) * 4, :], in_=sp,
                        func=AF.Exp)
                z6 = p_z.tile([128, 6, QGS], BF16, tag="z6", name="z6")
                nc.gpsimd.tensor_add(out=z6, in0=ek[:, 0:6, :],
                                     in1=ek[:, 6:12, :])
                z3 = p_z.tile([128, 3, QGS], BF16, tag="z3", name="z3")
                nc.vector.tensor_add(out=z3, in0=z6[:, 0:3, :],
                                     in1=z6[:, 3:6, :])
                zz = p_zr.tile([128, QGS], F32, tag="zz", name="zz")
                nc.vector.tensor_add(out=zz, in0=z3[:, 0, :], in1=z3[:, 1, :])
                nc.vector.tensor_add(out=zz, in0=zz, in1=z3[:, 2, :])
                rf = p_zr.tile([128, QGS], F32, tag="rf", name="rf")
                nc.vector.reciprocal_approx_fast(out=rf, in_=zz)
                rb = p_zr.tile([128, QGS], BF16, tag="rb", name="rb")
                nc.vector.tensor_copy(out=rb, in_=rf)
                # A = E * R (in place), split DVE / GPSIMD
                nc.vector.tensor_mul(out=ek[:, 0:6, :], in0=ek[:, 0:6, :],
                                     in1=_bcast_mid(rb, 6))
                nc.gpsimd.tensor_mul(out=ek[:, 6:12, :], in0=ek[:, 6:12, :],
                                     in1=_bcast_mid(rb, 6))
                # PSUM group start/stop tracking is per partition-range:
                # each col-group (pr) of a bank needs its own start and stop.
                for hd in range(H):
                    g, pr, ch = hd // 4, hd % 2, (hd // 2) % 2
                    mm = nc.tensor.matmul(
                        out=av[g][pr * 64:(pr + 1) * 64,
                                  ch * QGS:(ch + 1) * QGS],
                        lhsT=vn_tiles[kb][:, hd * 64:(hd + 1) * 64],
                        rhs=ek[:, (hd // 4) * 4 + SLOT[hd % 4], :],
                        start=(kb == 0 and hd % 4 in (0, 1)),
                        stop=(kb == NKB - 1 and hd % 4 in (2, 3)),
                        tile_position=(0, pr * 64),
                        # sim's group-protocol tracker mis-addresses
                        # partition-offset PSUM writes; data semantics
                        # (pending-zero) are tracked per-tensor and correct
                        skip_group_check=True)
                    if kb == 0:
                        if hd % 4 in (0, 1):
                            av_first[g][pr] = mm
                        else:
                            add_dep_helper(mm.ins, av_first[g][pr].ins,
                                           reason="psum grp start")
                    if kb == NKB - 1:
                        if hd % 4 in (0, 1):
                            av_last[g][pr] = mm
                        else:
                            add_dep_helper(mm.ins, av_last[g][pr].ins,
                                           reason="psum grp stop")
            # evict attention + residual -> x2
            for g in range(3):
                for ch in range(2):
                    p = 2 * g + ch
                    xqt = p_xq.tile([128, QGS], F32, tag="xq", name="xqt")
                    nc.sync.dma_start(
                        out=xqt, in_=xq[p * 128:(p + 1) * 128, q0:q0 + QGS])
                    nc.vector.scalar_tensor_tensor(
                        out=x2_tiles[p][:, q0:q0 + QGS],
                        in0=av[g][:, ch * QGS:(ch + 1) * QGS],
                        scalar=0.0, in1=xqt, op0=ALU.add, op1=ALU.add)

        def ln2_mlp_qg(qg):
            q0 = qg * QGS
            x2b = []
            for ci in range(CI):
                bt = p_x2b.tile([128, QGS], BF16, tag=f"x2b{ci}",
                                name=f"x2b{ci}")
                nc.vector.tensor_copy(out=bt,
                                      in_=x2_tiles[ci][:, q0:q0 + QGS])
                x2b.append(bt)
            sb2, mb2 = ln_stats_and_bcast(x2b, QGS)
            h2 = []
            for ci in range(CI):
                ht = p_h2.tile([128, QGS], BF16, tag=f"h2_{ci}",
                               name=f"h2_{qg}_{ci}")
                nc.gpsimd.tensor_mul(out=ht, in0=x2b[ci], in1=sb2)
                nc.vector.tensor_add(out=ht, in0=ht, in1=mb2)
                h2.append(ht)
            # fc1 + gelu: 12 psum tiles, 2 j-tiles each
            g1_tiles = []
            for jtp in range(DFF // 256):
                ps = p_sps.tile([128, 2 * QGS], F32, tag="sp",
                                name=f"f1ps{qg}_{jtp}")
                # one accumulation group spanning both halves of the bank
                firsts, lasts = [], []
                for half in range(2):
                    jt = jtp * 2 + half
                    for ci in range(CI):
                        mm = nc.tensor.matmul(
                            out=ps[:, half * QGS:(half + 1) * QGS],
                            lhsT=wf1_tiles[ci][:, jt * 128:(jt + 1) * 128],
                            rhs=h2[ci],
                            start=(half == 0 and ci == 0),
                            stop=(half == 1 and ci == CI - 1))
                        if ci == 0:
                            firsts.append(mm)
                        if ci == CI - 1:
                            lasts.append(mm)
                add_dep_helper(firsts[1].ins, firsts[0].ins,
                               reason="psum grp start")
                add_dep_helper(lasts[1].ins, lasts[0].ins,
                               reason="psum grp stop")
                g1 = p_g1.tile([128, 2, QGS], BF16, tag=f"g1_{jtp}",
                               name=f"g1_{qg}_{jtp}")
                if has_c1:
                    for half in range(2):
                        jt = jtp * 2 + half
                        nc.scalar.activation(
                            out=g1[:, half, :],
                            in_=ps[:, half * QGS:(half + 1) * QGS],
                            func=AF.Gelu, bias=c1_sb[:, jt:jt + 1])
                else:
                    nc.scalar.activation(
                        out=g1.rearrange("p a b -> p (a b)"), in_=ps,
                        func=AF.Gelu)
                g1_tiles.append(g1)
            # fc2: 6 out-tiles in 2 rounds of 3 (reuses "av" psum slots)
            for rnd in range(2):
                ps2 = [p_avps.tile([128, 512], F32, tag="av",
                                   name=f"f2ps{qg}_{rnd}_{i}")
                       for i in range(3)]
                for chunk in range(4):
                    wf2c = []
                    for i in range(6):
                        ci2 = chunk * 6 + i
                        wt = p_wf2.tile([128, D], BF16, tag="wf2",
                                        name="wf2t")
                        nc.sync.dma_start(
                            out=wt, in_=wfc2_d[ci2 * 128:(ci2 + 1) * 128, :])
                        wf2c.append(wt)
                    for i3 in range(3):
                        jt2 = rnd * 3 + i3
                        for i in range(6):
                            ci2 = chunk * 6 + i
                            jtp, half = ci2 // 2, ci2 % 2
                            nc.tensor.matmul(
                                out=ps2[i3][:, :QGS],
                                lhsT=wf2c[i][:, jt2 * 128:(jt2 + 1) * 128],
                                rhs=g1_tiles[jtp][:, half, :],
                                start=(chunk == 0 and i == 0),
                                stop=(chunk == 3 and i == 5))
                for i3 in range(3):
                    jt2 = rnd * 3 + i3
                    yt = p_y.tile([128, QGS], F32, tag="y", name="yt")
                    if has_c2:
                        nc.vector.scalar_tensor_tensor(
                            out=yt, in0=ps2[i3][:, :QGS],
                            scalar=c2_sb[:, jt2:jt2 + 1],
                            in1=x2_tiles[jt2][:, q0:q0 + QGS],
                            op0=ALU.add, op1=ALU.add)
                    else:
                        nc.vector.scalar_tensor_tensor(
                            out=yt, in0=ps2[i3][:, :QGS], scalar=0.0,
                            in1=x2_tiles[jt2][:, q0:q0 + QGS],
                            op0=ALU.add, op1=ALU.add)
                    nc.sync.dma_start(
                        out=out_t[jt2 * 128:(jt2 + 1) * 128, q0:q0 + QGS],
                        in_=yt)

        for qg in range(NQG):
            attention_qg(qg)
            ln2_mlp_qg(qg)


def _get_nc(has_cqkv, has_c1, has_c2):
    key = (has_cqkv, has_c1, has_c2)
    if key not in _CACHE:
        _CACHE[key] = build_nc(*key)
    return _CACHE[key]


def _prep_host(x, ln1_w, ln1_b, w_qkv, b_qkv, ln2_w, ln2_b, w_fc1, b_fc1,
               w_fc2, b_fc2):
    f32 = np.float32
    bf = ml_dtypes.bfloat16
    x = np.asarray(x, f32)
    ln1_w = np.asarray(ln1_w, f32); ln1_b = np.asarray(ln1_b, f32)
    ln2_w = np.asarray(ln2_w, f32); ln2_b = np.asarray(ln2_b, f32)
    w_qkv = np.asarray(w_qkv, f32); b_qkv = np.asarray(b_qkv, f32)
    w_fc1 = np.asarray(w_fc1, f32); b_fc1 = np.asarray(b_fc1, f32)
    w_fc2 = np.asarray(w_fc2, f32); b_fc2 = np.asarray(b_fc2, f32)

    scale = HD ** -0.5
    wq_eff = ln1_w[:, None] * w_qkv
    cqkv = ln1_b @ w_qkv + b_qkv
    wq_eff[:, :D] *= scale
    cqkv[:D] *= scale
    w1_eff = ln2_w[:, None] * w_fc1
    c1 = ln2_b @ w_fc1 + b_fc1
    c2 = b_fc2

    shared = {
        "w_qkv": np.ascontiguousarray(wq_eff.astype(bf)),
        "w_fc1": np.ascontiguousarray(w1_eff.astype(bf)),
        "w_fc2": np.ascontiguousarray(w_fc2.astype(bf)),
        "c_qkv": np.ascontiguousarray(cqkv.reshape(-1, 1)),
        "c_fc1": np.ascontiguousarray(c1.reshape(-1, 1)),
        "c_fc2": np.ascontiguousarray(c2.reshape(-1, 1)),
    }
    flags = (bool(np.any(cqkv)), bool(np.any(c1)), bool(np.any(c2)))

    in_maps = []
    for c in range(NCORES):
        b, qh = c // 2, c % 2
        xb = x[b]
        if qh:
            xb = np.concatenate([xb[TQ:], xb[:TQ]], axis=0)
        m = dict(shared)
        m["xt_bf"] = np.ascontiguousarray(xb.T.astype(bf))
        m["xq_f32"] = np.ascontiguousarray(xb[:TQ].T)
        in_maps.append(m)
    return in_maps, flags


def kernel(**inputs):
    global _LAST_RESULTS
    in_maps, flags = _prep_host(**inputs)
    nc = _get_nc(*flags)
    trace = bool(os.environ.get("KERNEL_TRACE"))
    res = run_bass_kernel_spmd(nc, in_maps, core_ids=list(range(NCORES)),
                               trace=trace)
    _LAST_RESULTS = res
    y = np.empty((B, P, D), np.float32)
    for c in range(NCORES):
        b, qh = c // 2, c % 2
        y[b, qh * TQ:(qh + 1) * TQ, :] = res.results[c]["out_t"].T
    return y


# revision 52
# speedup vs baseline: 175.6524x; 175.6524x over previous
"""Trainium2 Bass kernel for nn_Encoder_53274774340258.

One transformer encoder block (pre-norm), B=4 P=2048 D=768 H=12, with the
quirk that softmax normalizes over the HEAD axis (dim=1 of (B,H,P,P)).

Sharding: 8 cores = 4 batches x 2 query-halves. Each core computes K/V for
its whole batch (2048 tokens) and Q/attention/MLP for its 1024 queries.
No collectives. The host permutes tokens so each core's queries are always
columns 0:1024 of the shipped tensor (key order is irrelevant to attention),
keeping the compiled graph identical across cores (SPMD).

Layout: activations are kept transposed (feature-on-partition, token-on-free)
end to end; the host pre-transposes x and un-transposes the output. LN affine
params and the attention scale are folded into the weights on the host; LN
mean/var are computed on-device with ones-vector matmuls.
"""

import os
from contextlib import ExitStack

import ml_dtypes
import numpy as np

import concourse.bass as bass
import concourse.bacc as bacc
import concourse.mybir as mybir
import concourse.tile as tile
from concourse.bass_utils import run_bass_kernel_spmd
from concourse.tile import add_dep_helper

BF16 = mybir.dt.bfloat16
F32 = mybir.dt.float32
AF = mybir.ActivationFunctionType
ALU = mybir.AluOpType

B, P, D, H = 4, 2048, 768, 12
HD = D // H          # 64
DQKV = 3 * D         # 2304
DFF = 4 * D          # 3072
EPS = 1e-5
CI = D // 128        # 6 c-tiles
TK = P               # keys per core (full batch)
TQ = P // 2          # queries per core
NQG = 4              # query groups
QGS = TQ // NQG      # 256
NKB = TK // 128      # 16 key blocks
NCORES = 8

_CACHE = {}
_LAST_RESULTS = None


def _bcast_mid(ap2d, n):
    """View a (P, F) AP as (P, n, F) with a 0-step broadcast middle dim."""
    return bass.AP(
        tensor=ap2d.tensor,
        offset=ap2d.offset,
        ap=[ap2d.ap[0], [0, n], ap2d.ap[1]],
    )


def build_nc(has_cqkv, has_c1, has_c2):
    nc = bacc.Bacc()

    xt = nc.dram_tensor("xt_bf", [D, TK], BF16, kind="ExternalInput")
    xq = nc.dram_tensor("xq_f32", [D, TQ], F32, kind="ExternalInput")
    wqkv_d = nc.dram_tensor("w_qkv", [D, DQKV], BF16, kind="ExternalInput")
    wfc1_d = nc.dram_tensor("w_fc1", [D, DFF], BF16, kind="ExternalInput")
    wfc2_d = nc.dram_tensor("w_fc2", [DFF, D], BF16, kind="ExternalInput")
    cqkv_d = nc.dram_tensor("c_qkv", [DQKV, 1], F32, kind="ExternalInput")
    c1_d = nc.dram_tensor("c_fc1", [DFF, 1], F32, kind="ExternalInput")
    c2_d = nc.dram_tensor("c_fc2", [D, 1], F32, kind="ExternalInput")
    out_t = nc.dram_tensor("out_t", [D, TQ], F32, kind="ExternalOutput")

    with tile.TileContext(nc) as tc:
        _build(tc, nc, xt, xq, wqkv_d, wfc1_d, wfc2_d, cqkv_d, c1_d, c2_d,
               out_t, has_cqkv, has_c1, has_c2)
    nc.compile()
    return nc


def _build(tc, nc, xt, xq, wqkv_d, wfc1_d, wfc2_d, cqkv_d, c1_d, c2_d, out_t,
           has_cqkv, has_c1, has_c2):
    top = ExitStack()
    with top:
        pconst = top.enter_context(tc.tile_pool(name="pconst", bufs=1))
        ones_col = pconst.tile([128, 1], BF16)
        nc.vector.memset(ones_col, 1.0)
        ones_row = pconst.tile([1, 128], BF16)
        nc.vector.memset(ones_row, 1.0)
        eps_tile = pconst.tile([1, 1], F32)
        nc.vector.memset(eps_tile, EPS)

        cqkv_sb = c1_sb = c2_sb = None
        if has_cqkv:
            cqkv_sb = pconst.tile([128, DQKV // 128], F32)
            nc.sync.dma_start(
                out=cqkv_sb,
                in_=cqkv_d.rearrange("(j p) one -> p (j one)", p=128))
        if has_c1:
            c1_sb = pconst.tile([128, DFF // 128], F32)
            nc.sync.dma_start(
                out=c1_sb, in_=c1_d.rearrange("(j p) one -> p (j one)", p=128))
        if has_c2:
            c2_sb = pconst.tile([128, D // 128], F32)
            nc.sync.dma_start(
                out=c2_sb, in_=c2_d.rearrange("(j p) one -> p (j one)", p=128))

        # small PSUM pool shared by LN stats + broadcast matmuls (1 bank)
        p_small_ps = top.enter_context(
            tc.tile_pool(name="p_small_ps", bufs=1, space="PSUM"))
        p_srow = top.enter_context(tc.tile_pool(name="p_srow", bufs=1))

        def ln_stats_and_bcast(src_aps, n):
            """src_aps: CI (128, n) bf16 APs; returns (sb, mb): (128, n) bf16
            broadcast tiles for s = rsqrt(var+eps) and m = -mu*s."""
            sq = []
            for ci in range(CI):
                sqt = p_srow.tile([128, n], BF16, tag="sq", bufs=2,
                                  name=f"sq{ci}")
                nc.gpsimd.tensor_mul(out=sqt, in0=src_aps[ci],
                                     in1=src_aps[ci])
                sq.append(sqt)
            psx = p_small_ps.tile([1, n], F32, tag="smallps")
            psq = p_small_ps.tile([1, n], F32, tag="smallps")
            for ci in range(CI):
                nc.tensor.matmul(out=psx, lhsT=ones_col, rhs=src_aps[ci],
                                 start=(ci == 0), stop=(ci == CI - 1))
            for ci in range(CI):
                nc.tensor.matmul(out=psq, lhsT=ones_col, rhs=sq[ci],
                                 start=(ci == 0), stop=(ci == CI - 1))
            mu = p_srow.tile([1, n], F32, tag="mu")
            nc.scalar.activation(out=mu, in_=psx, func=AF.Identity,
                                 scale=1.0 / D)
            ex2 = p_srow.tile([1, n], F32, tag="ex2")
            nc.scalar.activation(out=ex2, in_=psq, func=AF.Identity,
                                 scale=1.0 / D)
            var = p_srow.tile([1, n], F32, tag="var")
            nc.vector.tensor_mul(out=var, in0=mu, in1=mu)
            nc.vector.tensor_sub(out=var, in0=ex2, in1=var)
            # rsqrt(var+eps) = exp(-0.5*ln(var+eps)); Ln/Exp share a table set
            lnv = p_srow.tile([1, n], F32, tag="lnv")
            nc.scalar.activation(out=lnv, in_=var, func=AF.Ln, bias=eps_tile)
            s_bf = p_srow.tile([1, n], BF16, tag="sbf")
            nc.scalar.activation(out=s_bf, in_=lnv, func=AF.Exp, scale=-0.5)
            m_bf = p_srow.tile([1, n], BF16, tag="mbf")
            nc.vector.scalar_tensor_tensor(
                out=m_bf, in0=mu, scalar=-1.0, in1=s_bf,
                op0=ALU.mult, op1=ALU.mult)
            psb = p_small_ps.tile([128, n], F32, tag="smallps")
            nc.tensor.matmul(out=psb, lhsT=ones_row, rhs=s_bf,
                             start=True, stop=True)
            sb = p_srow.tile([128, n], BF16, tag="sb", bufs=2)
            nc.scalar.activation(out=sb, in_=psb, func=AF.Copy)
            psb2 = p_small_ps.tile([128, n], F32, tag="smallps")
            nc.tensor.matmul(out=psb2, lhsT=ones_row, rhs=m_bf,
                             start=True, stop=True)
            mb = p_srow.tile([128, n], BF16, tag="mb", bufs=2)
            nc.scalar.activation(out=mb, in_=psb2, func=AF.Copy)
            return sb, mb

        # long-lived output pools of the QKV phase
        p_qt = top.enter_context(tc.tile_pool(name="p_qt", bufs=1))
        p_kt = top.enter_context(tc.tile_pool(name="p_kt", bufs=1))
        p_vn = top.enter_context(tc.tile_pool(name="p_vn", bufs=1))
        qt_tiles = [p_qt.tile([128, TQ], BF16, tag=f"qt{j}", name=f"qt{j}")
                    for j in range(CI)]
        kt_tiles = [p_kt.tile([128, TK], BF16, tag=f"kt{j}", name=f"kt{j}")
                    for j in range(CI)]
        vn_tiles = [p_vn.tile([128, D], BF16, tag=f"vn{t}", name=f"vn{t}")
                    for t in range(NKB)]

        # ---------------- Phases A-C: LN1 -> h -> QKV ----------------
        with tc.tile_pool(name="p_h", bufs=1) as p_h, \
             tc.tile_pool(name="p_x", bufs=1) as p_x, \
             tc.tile_pool(name="p_wqkv", bufs=1) as p_wqkv, \
             tc.tile_pool(name="p_cps", bufs=4, space="PSUM") as p_cps:
            xt_tiles, h_tiles, wq_tiles = [], [], []
            for ci in range(CI):
                xtt = p_x.tile([128, TK], BF16, tag=f"xt{ci}", name=f"xt{ci}")
                xt_tiles.append(xtt)
                ht = p_h.tile([128, TK], BF16, tag=f"h{ci}", name=f"h{ci}")
                h_tiles.append(ht)
                wt = p_wqkv.tile([128, DQKV], BF16, tag=f"wqkv{ci}",
                                 name=f"wqkv{ci}")
                nc.sync.dma_start(out=wt,
                                  in_=wqkv_d[ci * 128:(ci + 1) * 128, :])
                wq_tiles.append(wt)

            for tg in range(TK // 512):     # tg-major DMA: LN1 starts early
                for ci in range(CI):
                    nc.sync.dma_start(
                        out=xt_tiles[ci][:, tg * 512:(tg + 1) * 512],
                        in_=xt[ci * 128:(ci + 1) * 128,
                               tg * 512:(tg + 1) * 512])
            for tg in range(TK // 512):
                cs = slice(tg * 512, (tg + 1) * 512)
                sb, mb = ln_stats_and_bcast(
                    [t[:, cs] for t in xt_tiles], 512)
                for ci in range(CI):
                    nc.gpsimd.tensor_mul(out=h_tiles[ci][:, cs],
                                         in0=xt_tiles[ci][:, cs], in1=sb)
                    nc.vector.tensor_add(out=h_tiles[ci][:, cs],
                                         in0=h_tiles[ci][:, cs], in1=mb)

            def proj_T(j0, dst, cols0, ncols, bias_col):
                ps = p_cps.tile([128, 512], F32, tag="cps", name="cps")
                for ci in range(CI):
                    nc.tensor.matmul(
                        out=ps[:, :ncols],
                        lhsT=wq_tiles[ci][:, j0:j0 + 128],
                        rhs=h_tiles[ci][:, cols0:cols0 + ncols],
                        start=(ci == 0), stop=(ci == CI - 1))
                if has_cqkv:
                    nc.scalar.activation(
                        out=dst[:, cols0:cols0 + ncols], in_=ps[:, :ncols],
                        func=AF.Identity,
                        bias=cqkv_sb[:, bias_col:bias_col + 1])
                else:
                    nc.scalar.activation(
                        out=dst[:, cols0:cols0 + ncols], in_=ps[:, :ncols],
                        func=AF.Copy)

            for jt in range(CI):            # Q^T: local queries only
                for tt in range(TQ // 512):
                    proj_T(jt * 128, qt_tiles[jt], tt * 512, 512, jt)
            for tg in range(TK // 512):     # K^T: tg-outer so attention can
                for jt in range(CI):        # start as soon as kb 0..3 ready
                    proj_T(D + jt * 128, kt_tiles[jt], tg * 512, 512, CI + jt)
            # V natural: (token, dim) via lhsT = h block
            for tb in range(NKB):
                for (n0, nw) in ((0, 512), (512, 256)):
                    ps = p_cps.tile([128, 512], F32, tag="cps", name="cps")
                    for ci in range(CI):
                        nc.tensor.matmul(
                            out=ps[:, :nw],
                            lhsT=h_tiles[ci][:, tb * 128:(tb + 1) * 128],
                            rhs=wq_tiles[ci][:, 2 * D + n0:2 * D + n0 + nw],
                            start=(ci == 0), stop=(ci == CI - 1))
                    nc.vector.tensor_copy(
                        out=vn_tiles[tb][:, n0:n0 + nw], in_=ps[:, :nw])

        # ---------------- Phase D/E pools ----------------
        p_sps = top.enter_context(
            tc.tile_pool(name="p_sps", bufs=2, space="PSUM"))
        p_avps = top.enter_context(
            tc.tile_pool(name="p_avps", bufs=3, space="PSUM"))
        p_e = top.enter_context(tc.tile_pool(name="p_e", bufs=3))
        p_z = top.enter_context(tc.tile_pool(name="p_z", bufs=2))
        p_zr = top.enter_context(tc.tile_pool(name="p_zr", bufs=2))
        p_x2 = top.enter_context(tc.tile_pool(name="p_x2", bufs=1))
        p_xq = top.enter_context(tc.tile_pool(name="p_xq", bufs=1))
        p_wf1 = top.enter_context(tc.tile_pool(name="p_wf1", bufs=1))
        p_h2 = top.enter_context(tc.tile_pool(name="p_h2", bufs=1))
        p_g1 = top.enter_context(tc.tile_pool(name="p_g1", bufs=1))
        p_wf2 = top.enter_context(tc.tile_pool(name="p_wf2", bufs=2))
        p_y = top.enter_context(tc.tile_pool(name="p_y", bufs=2))
        p_x2b = top.enter_context(tc.tile_pool(name="p_x2b", bufs=1))

        x2_tiles = [p_x2.tile([128, TQ], F32, tag=f"x2_{j}", name=f"x2_{j}")
                    for j in range(CI)]
        wf1_tiles = []
        for ci in range(CI):
            wt = p_wf1.tile([128, DFF], BF16, tag=f"wfc1{ci}",
                            name=f"wfc1{ci}")
            nc.sync.dma_start(out=wt, in_=wfc1_d[ci * 128:(ci + 1) * 128, :])
            wf1_tiles.append(wt)

        def attention_qg(qg):
            q0 = qg * QGS
            av = [p_avps.tile([128, 512], F32, tag="av", name=f"av{qg}_{g}")
                  for g in range(3)]
            av_first = [[None, None] for _ in range(3)]
            av_last = [[None, None] for _ in range(3)]
            for kb in range(NKB):
                ek = p_e.tile([128, H, QGS], BF16, tag="ek",
                              name=f"ek{qg}_{kb}")
                # Concurrent row-group MMs must NOT share a PSUM bank
                # (HW fault) -> map paired heads to different banks:
                # slot order in sp/ek is [h0, h2, h1, h3] of each quad.
                SLOT = (0, 2, 1, 3)
                for pg in range(3):       # 4 heads per scores tile
                    sp = p_sps.tile([128, 4 * QGS], F32, tag="sp",
                                    name=f"sp{qg}_{kb}_{pg}")
                    mms = []
                    for hh in range(4):
                        hd = pg * 4 + hh
                        jt, pr = hd // 2, hd % 2
                        sl = SLOT[hh]
                        mm = nc.tensor.matmul(
                            out=sp[:, sl * QGS:(sl + 1) * QGS],
                            lhsT=kt_tiles[jt][pr * 64:(pr + 1) * 64,
                                              kb * 128:(kb + 1) * 128],
                            rhs=qt_tiles[jt][pr * 64:(pr + 1) * 64,
                                             q0:q0 + QGS],
                            start=(hh in (0, 1)), stop=(hh in (2, 3)),
                            tile_position=(pr * 64, 0))
                        mms.append(mm)
                    # each bank's stop MM must run after its start MM
                    add_dep_helper(mms[2].ins, mms[0].ins, reason="psum grp")
                    add_dep_helper(mms[3].ins, mms[1].ins, reason="psum grp")
                    nc.scalar.activation(
                        out=ek[:, pg * 4:(pg + 1) * 4, :], in_=sp,
                        func=AF.Exp)
                z6 = p_z.tile([128, 6, QGS], BF16, tag="z6", name="z6")
                nc.gpsimd.tensor_add(out=z6, in0=ek[:, 0:6, :],
                                     in1=ek[:, 6:12, :])
                z3 = p_z.tile([128, 3, QGS], BF16, tag="z3", name="z3")
                nc.vector.tensor_add(out=z3, in0=z6[:, 0:3, :],
                                     in1=z6[:, 3:6, :])
                zz = p_zr.tile([128, QGS], F32, tag="zz", name="zz")
                nc.vector.tensor_add(out=zz, in0=z3[:, 0, :], in1=z3[:, 1, :])
                nc.vector.tensor_add(out=zz, in0=zz, in1=z3[:, 2, :])
                rf = p_zr.tile([128, QGS], F32, tag="rf", name="rf")
                nc.vector.reciprocal_approx_fast(out=rf, in_=zz)
                rb = p_zr.tile([128, QGS], BF16, tag="rb", name="rb")
                nc.vector.tensor_copy(out=rb, in_=rf)
                # A = E * R (in place), split DVE / GPSIMD
                nc.vector.tensor_mul(out=ek[:, 0:6, :], in0=ek[:, 0:6, :],
                                     in1=_bcast_mid(rb, 6))
                nc.gpsimd.tensor_mul(out=ek[:, 6:12, :], in0=ek[:, 6:12, :],
                                     in1=_bcast_mid(rb, 6))
                # PSUM group start/stop tracking is per partition-range:
                # each col-group (pr) of a bank needs its own start and stop.
                for hd in range(H):
                    g, pr, ch = hd // 4, hd % 2, (hd // 2) % 2
                    mm = nc.tensor.matmul(
                        out=av[g][pr * 64:(pr + 1) * 64,
                                  ch * QGS:(ch + 1) * QGS],
                        lhsT=vn_tiles[kb][:, hd * 64:(hd + 1) * 64],
                        rhs=ek[:, (hd // 4) * 4 + SLOT[hd % 4], :],
                        start=(kb == 0 and hd % 4 in (0, 1)),
                        stop=(kb == NKB - 1 and hd % 4 in (2, 3)),
                        tile_position=(0, pr * 64),
                        # sim's group-protocol tracker mis-addresses
                        # partition-offset PSUM writes; data semantics
                        # (pending-zero) are tracked per-tensor and correct
                        skip_group_check=True)
                    if kb == 0:
                        if hd % 4 in (0, 1):
                            av_first[g][pr] = mm
                        else:
                            add_dep_helper(mm.ins, av_first[g][pr].ins,
                                           reason="psum grp start")
                    if kb == NKB - 1:
                        if hd % 4 in (0, 1):
                            av_last[g][pr] = mm
                        else:
                            add_dep_helper(mm.ins, av_last[g][pr].ins,
                                           reason="psum grp stop")
            # evict attention + residual -> x2 (one DMA for all 6 row-blocks)
            xqt = p_xq.tile([128, CI, QGS], F32, tag="xq", name="xqt")
            nc.sync.dma_start(
                out=xqt,
                in_=xq[:, q0:q0 + QGS].rearrange("(a p) q -> p a q", p=128))
            for g in range(3):
                for ch in range(2):
                    p = 2 * g + ch
                    nc.vector.scalar_tensor_tensor(
                        out=x2_tiles[p][:, q0:q0 + QGS],
                        in0=av[g][:, ch * QGS:(ch + 1) * QGS],
                        scalar=0.0, in1=xqt[:, p, :], op0=ALU.add, op1=ALU.add)

        def ln2_mlp_qg(qg):
            q0 = qg * QGS
            x2b = []
            for ci in range(CI):
                bt = p_x2b.tile([128, QGS], BF16, tag=f"x2b{ci}",
                                name=f"x2b{ci}")
                nc.vector.tensor_copy(out=bt,
                                      in_=x2_tiles[ci][:, q0:q0 + QGS])
                x2b.append(bt)
            sb2, mb2 = ln_stats_and_bcast(x2b, QGS)
            h2 = []
            for ci in range(CI):
                ht = p_h2.tile([128, QGS], BF16, tag=f"h2_{ci}",
                               name=f"h2_{qg}_{ci}")
                nc.gpsimd.tensor_mul(out=ht, in0=x2b[ci], in1=sb2)
                nc.vector.tensor_add(out=ht, in0=ht, in1=mb2)
                h2.append(ht)
            # fc1 + gelu: 6 psum tiles (2 banks), 4 j-tiles each
            g1_tiles = []
            for jtq in range(DFF // 512):
                ps = p_sps.tile([128, 4, QGS], F32, tag="sp",
                                name=f"f1ps{qg}_{jtq}")
                # one accumulation group per bank (2 slots each)
                firsts, lasts = {}, {}
                for sl in range(4):
                    jt = jtq * 4 + sl
                    for ci in range(CI):
                        mm = nc.tensor.matmul(
                            out=ps[:, sl, :],
                            lhsT=wf1_tiles[ci][:, jt * 128:(jt + 1) * 128],
                            rhs=h2[ci],
                            start=(sl % 2 == 0 and ci == 0),
                            stop=(sl % 2 == 1 and ci == CI - 1),
                            skip_group_check=True)
                        if ci == 0:
                            firsts[sl] = mm
                        if ci == CI - 1:
                            lasts[sl] = mm
                for a, b in ((0, 1), (2, 3)):
                    add_dep_helper(firsts[b].ins, firsts[a].ins, reason="grp")
                    add_dep_helper(lasts[b].ins, lasts[a].ins, reason="grp")
                g1 = p_g1.tile([128, 4, QGS], BF16, tag=f"g1_{jtq}",
                               name=f"g1_{qg}_{jtq}")
                if has_c1:
                    for sl in range(4):
                        jt = jtq * 4 + sl
                        nc.scalar.activation(
                            out=g1[:, sl, :], in_=ps[:, sl, :],
                            func=AF.Gelu, bias=c1_sb[:, jt:jt + 1])
                else:
                    nc.scalar.activation(
                        out=g1.rearrange("p a b -> p (a b)"),
                        in_=ps.rearrange("p a b -> p (a b)"),
                        func=AF.Gelu)
                g1_tiles.append(g1)
            # fc2: single pass over wfc2; 6 out-tiles packed into 2 "sp"
            # psum tiles, one accumulation group per bank (2 slots each)
            psA = p_sps.tile([128, 4 * QGS], F32, tag="sp", name=f"f2A{qg}")
            psB = p_sps.tile([128, 4 * QGS], F32, tag="sp", name=f"f2B{qg}")
            slot_of = [(psA, 0), (psA, 1), (psA, 2), (psA, 3),
                       (psB, 0), (psB, 1)]
            firsts2, lasts2 = {}, {}
            for chunk in range(8):   # 3 k-tiles per DMA
                wt = p_wf2.tile([128, 3, D], BF16, tag="wf2", name="wf2t")
                nc.sync.dma_start(
                    out=wt,
                    in_=wfc2_d[chunk * 384:(chunk + 1) * 384, :]
                    .rearrange("(a p) d -> p a d", p=128))
                for jt2 in range(6):
                    pst, sl = slot_of[jt2]
                    for i in range(3):
                        ci2 = chunk * 3 + i
                        jtq, slq = ci2 // 4, ci2 % 4
                        mm = nc.tensor.matmul(
                            out=pst[:, sl * QGS:(sl + 1) * QGS],
                            lhsT=wt[:, i, jt2 * 128:(jt2 + 1) * 128],
                            rhs=g1_tiles[jtq][:, slq, :],
                            start=(chunk == 0 and i == 0 and sl % 2 == 0),
                            stop=(chunk == 7 and i == 2 and
                                  (sl % 2 == 1 or jt2 == 5)),
                            skip_group_check=True)
                        if chunk == 0 and i == 0:
                            firsts2[jt2] = mm
                        if chunk == 7 and i == 2:
                            lasts2[jt2] = mm
            for a, b in ((0, 1), (2, 3), (4, 5)):
                add_dep_helper(firsts2[b].ins, firsts2[a].ins, reason="grp")
                add_dep_helper(lasts2[b].ins, lasts2[a].ins, reason="grp")
            for jt2 in range(6):
                pst, sl = slot_of[jt2]
                yt = p_y.tile([128, QGS], F32, tag="y", name="yt")
                if has_c2:
                    nc.vector.scalar_tensor_tensor(
                        out=yt, in0=pst[:, sl * QGS:(sl + 1) * QGS],
                        scalar=c2_sb[:, jt2:jt2 + 1],
                        in1=x2_tiles[jt2][:, q0:q0 + QGS],
                        op0=ALU.add, op1=ALU.add)
                else:
                    nc.vector.scalar_tensor_tensor(
                        out=yt, in0=pst[:, sl * QGS:(sl + 1) * QGS],
                        scalar=0.0,
                        in1=x2_tiles[jt2][:, q0:q0 + QGS],
                        op0=ALU.add, op1=ALU.add)
                nc.gpsimd.dma_start(
                    out=out_t[jt2 * 128:(jt2 + 1) * 128, q0:q0 + QGS],
                    in_=yt)

        for qg in range(NQG):
            attention_qg(qg)
            ln2_mlp_qg(qg)


def _get_nc(has_cqkv, has_c1, has_c2):
    key = (has_cqkv, has_c1, has_c2)
    if key not in _CACHE:
        _CACHE[key] = build_nc(*key)
    return _CACHE[key]


def _prep_host(x, ln1_w, ln1_b, w_qkv, b_qkv, ln2_w, ln2_b, w_fc1, b_fc1,
               w_fc2, b_fc2):
    f32 = np.float32
    bf = ml_dtypes.bfloat16
    x = np.asarray(x, f32)
    ln1_w = np.asarray(ln1_w, f32); ln1_b = np.asarray(ln1_b, f32)
    ln2_w = np.asarray(ln2_w, f32); ln2_b = np.asarray(ln2_b, f32)
    w_qkv = np.asarray(w_qkv, f32); b_qkv = np.asarray(b_qkv, f32)
    w_fc1 = np.asarray(w_fc1, f32); b_fc1 = np.asarray(b_fc1, f32)
    w_fc2 = np.asarray(w_fc2, f32); b_fc2 = np.asarray(b_fc2, f32)

    scale = HD ** -0.5
    wq_eff = ln1_w[:, None] * w_qkv
    cqkv = ln1_b @ w_qkv + b_qkv
    wq_eff[:, :D] *= scale
    cqkv[:D] *= scale
    w1_eff = ln2_w[:, None] * w_fc1
    c1 = ln2_b @ w_fc1 + b_fc1
    c2 = b_fc2

    shared = {
        "w_qkv": np.ascontiguousarray(wq_eff.astype(bf)),
        "w_fc1": np.ascontiguousarray(w1_eff.astype(bf)),
        "w_fc2": np.ascontiguousarray(w_fc2.astype(bf)),
        "c_qkv": np.ascontiguousarray(cqkv.reshape(-1, 1)),
        "c_fc1": np.ascontiguousarray(c1.reshape(-1, 1)),
        "c_fc2": np.ascontiguousarray(c2.reshape(-1, 1)),
    }
    flags = (bool(np.any(cqkv)), bool(np.any(c1)), bool(np.any(c2)))

    in_maps = []
    for c in range(NCORES):
        b, qh = c // 2, c % 2
        xb = x[b]
        if qh:
            xb = np.concatenate([xb[TQ:], xb[:TQ]], axis=0)
        m = dict(shared)
        m["xt_bf"] = np.ascontiguousarray(xb.T.astype(bf))
        m["xq_f32"] = np.ascontiguousarray(xb[:TQ].T)
        in_maps.append(m)
    return in_maps, flags


def kernel(**inputs):
    global _LAST_RESULTS
    in_maps, flags = _prep_host(**inputs)
    nc = _get_nc(*flags)
    trace = bool(os.environ.get("KERNEL_TRACE"))
    try:
        res = run_bass_kernel_spmd(nc, in_maps, core_ids=list(range(NCORES)),
                                   trace=trace)
    except ModuleNotFoundError:
        # NTFF profiling hook unavailable in this axon client
        res = run_bass_kernel_spmd(nc, in_maps, core_ids=list(range(NCORES)),
                                   trace=False)
    _LAST_RESULTS = res
    y = np.empty((B, P, D), np.float32)
    for c in range(NCORES):
        b, qh = c // 2, c % 2
        y[b, qh * TQ:(qh + 1) * TQ, :] = res.results[c]["out_t"].T
    return y


# revision 54
# speedup vs baseline: 175.7519x; 1.0006x over previous
"""Trainium2 Bass kernel for nn_Encoder_53274774340258.

One transformer encoder block (pre-norm), B=4 P=2048 D=768 H=12, with the
quirk that softmax normalizes over the HEAD axis (dim=1 of (B,H,P,P)).

Sharding: 8 cores = 4 batches x 2 query-halves. Each core computes K/V for
its whole batch (2048 tokens) and Q/attention/MLP for its 1024 queries.
No collectives. The host permutes tokens so each core's queries are always
columns 0:1024 of the shipped tensor (key order is irrelevant to attention),
keeping the compiled graph identical across cores (SPMD).

Layout: activations are kept transposed (feature-on-partition, token-on-free)
end to end; the host pre-transposes x and un-transposes the output. LN affine
params and the attention scale are folded into the weights on the host; LN
mean/var are computed on-device with ones-vector matmuls.
"""

import os
from contextlib import ExitStack

import ml_dtypes
import numpy as np

import concourse.bass as bass
import concourse.bacc as bacc
import concourse.mybir as mybir
import concourse.tile as tile
from concourse.bass_utils import run_bass_kernel_spmd
from concourse.tile import add_dep_helper

BF16 = mybir.dt.bfloat16
F32 = mybir.dt.float32
AF = mybir.ActivationFunctionType
ALU = mybir.AluOpType

B, P, D, H = 4, 2048, 768, 12
HD = D // H          # 64
DQKV = 3 * D         # 2304
DFF = 4 * D          # 3072
EPS = 1e-5
CI = D // 128        # 6 c-tiles
TK = P               # keys per core (full batch)
TQ = P // 2          # queries per core
NQG = 4              # query groups
QGS = TQ // NQG      # 256
NKB = TK // 128      # 16 key blocks
NCORES = 8

_CACHE = {}
_LAST_RESULTS = None


def _bcast_mid(ap2d, n):
    """View a (P, F) AP as (P, n, F) with a 0-step broadcast middle dim."""
    return bass.AP(
        tensor=ap2d.tensor,
        offset=ap2d.offset,
        ap=[ap2d.ap[0], [0, n], ap2d.ap[1]],
    )


def build_nc(has_cqkv, has_c1, has_c2):
    nc = bacc.Bacc()

    xt = nc.dram_tensor("xt_bf", [D, TK], BF16, kind="ExternalInput")
    xq = nc.dram_tensor("xq_f32", [D, TQ], F32, kind="ExternalInput")
    wqkv_d = nc.dram_tensor("w_qkv", [D, DQKV], BF16, kind="ExternalInput")
    wfc1_d = nc.dram_tensor("w_fc1", [D, DFF], BF16, kind="ExternalInput")
    wfc2_d = nc.dram_tensor("w_fc2", [DFF, D], BF16, kind="ExternalInput")
    cqkv_d = nc.dram_tensor("c_qkv", [DQKV, 1], F32, kind="ExternalInput")
    c1_d = nc.dram_tensor("c_fc1", [DFF, 1], F32, kind="ExternalInput")
    c2_d = nc.dram_tensor("c_fc2", [D, 1], F32, kind="ExternalInput")
    out_t = nc.dram_tensor("out_t", [D, TQ], F32, kind="ExternalOutput")

    with tile.TileContext(nc) as tc:
        _build(tc, nc, xt, xq, wqkv_d, wfc1_d, wfc2_d, cqkv_d, c1_d, c2_d,
               out_t, has_cqkv, has_c1, has_c2)
    nc.compile()
    return nc


def _build(tc, nc, xt, xq, wqkv_d, wfc1_d, wfc2_d, cqkv_d, c1_d, c2_d, out_t,
           has_cqkv, has_c1, has_c2):
    top = ExitStack()
    with top:
        pconst = top.enter_context(tc.tile_pool(name="pconst", bufs=1))
        ones_col = pconst.tile([128, 1], BF16)
        nc.vector.memset(ones_col, 1.0)
        ones_row = pconst.tile([1, 128], BF16)
        nc.vector.memset(ones_row, 1.0)
        eps_tile = pconst.tile([1, 1], F32)
        nc.vector.memset(eps_tile, EPS)

        cqkv_sb = c1_sb = c2_sb = None
        if has_cqkv:
            cqkv_sb = pconst.tile([128, DQKV // 128], F32)
            nc.sync.dma_start(
                out=cqkv_sb,
                in_=cqkv_d.rearrange("(j p) one -> p (j one)", p=128))
        if has_c1:
            c1_sb = pconst.tile([128, DFF // 128], F32)
            nc.sync.dma_start(
                out=c1_sb, in_=c1_d.rearrange("(j p) one -> p (j one)", p=128))
        if has_c2:
            c2_sb = pconst.tile([128, D // 128], F32)
            nc.sync.dma_start(
                out=c2_sb, in_=c2_d.rearrange("(j p) one -> p (j one)", p=128))

        # small PSUM pool shared by LN stats + broadcast matmuls (1 bank)
        p_small_ps = top.enter_context(
            tc.tile_pool(name="p_small_ps", bufs=1, space="PSUM"))
        p_srow = top.enter_context(tc.tile_pool(name="p_srow", bufs=1))

        def ln_stats_and_bcast(src_aps, n):
            """src_aps: CI (128, n) bf16 APs; returns (sb, mb): (128, n) bf16
            broadcast tiles for s = rsqrt(var+eps) and m = -mu*s."""
            sq = []
            for ci in range(CI):
                sqt = p_srow.tile([128, n], BF16, tag="sq", bufs=2,
                                  name=f"sq{ci}")
                nc.gpsimd.tensor_mul(out=sqt, in0=src_aps[ci],
                                     in1=src_aps[ci])
                sq.append(sqt)
            psx = p_small_ps.tile([1, n], F32, tag="smallps")
            psq = p_small_ps.tile([1, n], F32, tag="smallps")
            for ci in range(CI):
                nc.tensor.matmul(out=psx, lhsT=ones_col, rhs=src_aps[ci],
                                 start=(ci == 0), stop=(ci == CI - 1))
            for ci in range(CI):
                nc.tensor.matmul(out=psq, lhsT=ones_col, rhs=sq[ci],
                                 start=(ci == 0), stop=(ci == CI - 1))
            mu = p_srow.tile([1, n], F32, tag="mu")
            nc.scalar.activation(out=mu, in_=psx, func=AF.Identity,
                                 scale=1.0 / D)
            ex2 = p_srow.tile([1, n], F32, tag="ex2")
            nc.scalar.activation(out=ex2, in_=psq, func=AF.Identity,
                                 scale=1.0 / D)
            var = p_srow.tile([1, n], F32, tag="var")
            nc.vector.tensor_mul(out=var, in0=mu, in1=mu)
            nc.vector.tensor_sub(out=var, in0=ex2, in1=var)
            # rsqrt(var+eps) = exp(-0.5*ln(var+eps)); Ln/Exp share a table set
            lnv = p_srow.tile([1, n], F32, tag="lnv")
            nc.scalar.activation(out=lnv, in_=var, func=AF.Ln, bias=eps_tile)
            s_bf = p_srow.tile([1, n], BF16, tag="sbf")
            nc.scalar.activation(out=s_bf, in_=lnv, func=AF.Exp, scale=-0.5)
            m_bf = p_srow.tile([1, n], BF16, tag="mbf")
            nc.vector.scalar_tensor_tensor(
                out=m_bf, in0=mu, scalar=-1.0, in1=s_bf,
                op0=ALU.mult, op1=ALU.mult)
            psb = p_small_ps.tile([128, n], F32, tag="smallps")
            nc.tensor.matmul(out=psb, lhsT=ones_row, rhs=s_bf,
                             start=True, stop=True)
            sb = p_srow.tile([128, n], BF16, tag="sb", bufs=2)
            nc.scalar.activation(out=sb, in_=psb, func=AF.Copy)
            psb2 = p_small_ps.tile([128, n], F32, tag="smallps")
            nc.tensor.matmul(out=psb2, lhsT=ones_row, rhs=m_bf,
                             start=True, stop=True)
            mb = p_srow.tile([128, n], BF16, tag="mb", bufs=2)
            nc.scalar.activation(out=mb, in_=psb2, func=AF.Copy)
            return sb, mb

        # long-lived output pools of the QKV phase
        p_qt = top.enter_context(tc.tile_pool(name="p_qt", bufs=1))
        p_kt = top.enter_context(tc.tile_pool(name="p_kt", bufs=1))
        p_vn = top.enter_context(tc.tile_pool(name="p_vn", bufs=1))
        qt_tiles = [p_qt.tile([128, TQ], BF16, tag=f"qt{j}", name=f"qt{j}")
                    for j in range(CI)]
        kt_tiles = [p_kt.tile([128, TK], BF16, tag=f"kt{j}", name=f"kt{j}")
                    for j in range(CI)]
        vn_tiles = [p_vn.tile([128, D], BF16, tag=f"vn{t}", name=f"vn{t}")
                    for t in range(NKB)]

        # ---------------- Phases A-C: LN1 -> h -> QKV ----------------
        with tc.tile_pool(name="p_h", bufs=1) as p_h, \
             tc.tile_pool(name="p_x", bufs=1) as p_x, \
             tc.tile_pool(name="p_wqkv", bufs=1) as p_wqkv, \
             tc.tile_pool(name="p_cps", bufs=4, space="PSUM") as p_cps:
            xt_tiles, h_tiles, wq_tiles = [], [], []
            for ci in range(CI):
                xtt = p_x.tile([128, TK], BF16, tag=f"xt{ci}", name=f"xt{ci}")
                xt_tiles.append(xtt)
                ht = p_h.tile([128, TK], BF16, tag=f"h{ci}", name=f"h{ci}")
                h_tiles.append(ht)
                wt = p_wqkv.tile([128, DQKV], BF16, tag=f"wqkv{ci}",
                                 name=f"wqkv{ci}")
                nc.sync.dma_start(out=wt,
                                  in_=wqkv_d[ci * 128:(ci + 1) * 128, :])
                wq_tiles.append(wt)

            for tg in range(TK // 512):     # tg-major DMA: LN1 starts early
                for ci in range(CI):
                    nc.sync.dma_start(
                        out=xt_tiles[ci][:, tg * 512:(tg + 1) * 512],
                        in_=xt[ci * 128:(ci + 1) * 128,
                               tg * 512:(tg + 1) * 512])
            for tg in range(TK // 512):
                cs = slice(tg * 512, (tg + 1) * 512)
                sb, mb = ln_stats_and_bcast(
                    [t[:, cs] for t in xt_tiles], 512)
                for ci in range(CI):
                    nc.gpsimd.tensor_mul(out=h_tiles[ci][:, cs],
                                         in0=xt_tiles[ci][:, cs], in1=sb)
                    nc.vector.tensor_add(out=h_tiles[ci][:, cs],
                                         in0=h_tiles[ci][:, cs], in1=mb)

            def proj_T(j0, dst, cols0, ncols, bias_col):
                ps = p_cps.tile([128, 512], F32, tag="cps", name="cps")
                for ci in range(CI):
                    nc.tensor.matmul(
                        out=ps[:, :ncols],
                        lhsT=wq_tiles[ci][:, j0:j0 + 128],
                        rhs=h_tiles[ci][:, cols0:cols0 + ncols],
                        start=(ci == 0), stop=(ci == CI - 1))
                if has_cqkv:
                    nc.scalar.activation(
                        out=dst[:, cols0:cols0 + ncols], in_=ps[:, :ncols],
                        func=AF.Identity,
                        bias=cqkv_sb[:, bias_col:bias_col + 1])
                else:
                    nc.vector.tensor_copy(
                        out=dst[:, cols0:cols0 + ncols], in_=ps[:, :ncols])

            for jt in range(CI):            # Q^T: local queries only
                for tt in range(TQ // 512):
                    proj_T(jt * 128, qt_tiles[jt], tt * 512, 512, jt)
            for tg in range(TK // 512):     # K^T: tg-outer so attention can
                for jt in range(CI):        # start as soon as kb 0..3 ready
                    proj_T(D + jt * 128, kt_tiles[jt], tg * 512, 512, CI + jt)
            # V natural: (token, dim) via lhsT = h block
            for tb in range(NKB):
                for (n0, nw) in ((0, 512), (512, 256)):
                    ps = p_cps.tile([128, 512], F32, tag="cps", name="cps")
                    for ci in range(CI):
                        nc.tensor.matmul(
                            out=ps[:, :nw],
                            lhsT=h_tiles[ci][:, tb * 128:(tb + 1) * 128],
                            rhs=wq_tiles[ci][:, 2 * D + n0:2 * D + n0 + nw],
                            start=(ci == 0), stop=(ci == CI - 1))
                    nc.vector.tensor_copy(
                        out=vn_tiles[tb][:, n0:n0 + nw], in_=ps[:, :nw])

        # ---------------- Phase D/E pools ----------------
        p_sps = top.enter_context(
            tc.tile_pool(name="p_sps", bufs=2, space="PSUM"))
        p_avps = top.enter_context(
            tc.tile_pool(name="p_avps", bufs=3, space="PSUM"))
        p_e = top.enter_context(tc.tile_pool(name="p_e", bufs=3))
        p_z = top.enter_context(tc.tile_pool(name="p_z", bufs=2))
        p_zr = top.enter_context(tc.tile_pool(name="p_zr", bufs=2))
        p_x2 = top.enter_context(tc.tile_pool(name="p_x2", bufs=1))
        p_xq = top.enter_context(tc.tile_pool(name="p_xq", bufs=1))
        p_wf1 = top.enter_context(tc.tile_pool(name="p_wf1", bufs=1))
        p_h2 = top.enter_context(tc.tile_pool(name="p_h2", bufs=1))
        p_g1 = top.enter_context(tc.tile_pool(name="p_g1", bufs=1))
        p_wf2 = top.enter_context(tc.tile_pool(name="p_wf2", bufs=2))
        p_y = top.enter_context(tc.tile_pool(name="p_y", bufs=2))
        p_x2b = top.enter_context(tc.tile_pool(name="p_x2b", bufs=1))

        x2_tiles = [p_x2.tile([128, TQ], F32, tag=f"x2_{j}", name=f"x2_{j}")
                    for j in range(CI)]
        wf1_tiles = []
        for ci in range(CI):
            wt = p_wf1.tile([128, DFF], BF16, tag=f"wfc1{ci}",
                            name=f"wfc1{ci}")
            nc.sync.dma_start(out=wt, in_=wfc1_d[ci * 128:(ci + 1) * 128, :])
            wf1_tiles.append(wt)

        def attention_qg(qg):
            q0 = qg * QGS
            av = [p_avps.tile([128, 512], F32, tag="av", name=f"av{qg}_{g}")
                  for g in range(3)]
            av_first = [[None, None] for _ in range(3)]
            av_last = [[None, None] for _ in range(3)]
            for kb in range(NKB):
                ek = p_e.tile([128, H, QGS], BF16, tag="ek",
                              name=f"ek{qg}_{kb}")
                # Concurrent row-group MMs must NOT share a PSUM bank
                # (HW fault) -> map paired heads to different banks:
                # slot order in sp/ek is [h0, h2, h1, h3] of each quad.
                SLOT = (0, 2, 1, 3)
                for pg in range(3):       # 4 heads per scores tile
                    sp = p_sps.tile([128, 4 * QGS], F32, tag="sp",
                                    name=f"sp{qg}_{kb}_{pg}")
                    mms = []
                    for hh in range(4):
                        hd = pg * 4 + hh
                        jt, pr = hd // 2, hd % 2
                        sl = SLOT[hh]
                        mm = nc.tensor.matmul(
                            out=sp[:, sl * QGS:(sl + 1) * QGS],
                            lhsT=kt_tiles[jt][pr * 64:(pr + 1) * 64,
                                              kb * 128:(kb + 1) * 128],
                            rhs=qt_tiles[jt][pr * 64:(pr + 1) * 64,
                                             q0:q0 + QGS],
                            start=(hh in (0, 1)), stop=(hh in (2, 3)),
                            tile_position=(pr * 64, 0))
                        mms.append(mm)
                    # each bank's stop MM must run after its start MM
                    add_dep_helper(mms[2].ins, mms[0].ins, reason="psum grp")
                    add_dep_helper(mms[3].ins, mms[1].ins, reason="psum grp")
                    nc.scalar.activation(
                        out=ek[:, pg * 4:(pg + 1) * 4, :], in_=sp,
                        func=AF.Exp)
                z6 = p_z.tile([128, 6, QGS], BF16, tag="z6", name="z6")
                nc.gpsimd.tensor_add(out=z6, in0=ek[:, 0:6, :],
                                     in1=ek[:, 6:12, :])
                z3 = p_z.tile([128, 3, QGS], BF16, tag="z3", name="z3")
                nc.vector.tensor_add(out=z3, in0=z6[:, 0:3, :],
                                     in1=z6[:, 3:6, :])
                zz = p_zr.tile([128, QGS], F32, tag="zz", name="zz")
                nc.vector.tensor_add(out=zz, in0=z3[:, 0, :], in1=z3[:, 1, :])
                nc.vector.tensor_add(out=zz, in0=zz, in1=z3[:, 2, :])
                rf = p_zr.tile([128, QGS], F32, tag="rf", name="rf")
                nc.vector.reciprocal_approx_fast(out=rf, in_=zz)
                rb = p_zr.tile([128, QGS], BF16, tag="rb", name="rb")
                nc.vector.tensor_copy(out=rb, in_=rf)
                # A = E * R (in place), split DVE / GPSIMD
                nc.vector.tensor_mul(out=ek[:, 0:6, :], in0=ek[:, 0:6, :],
                                     in1=_bcast_mid(rb, 6))
                nc.gpsimd.tensor_mul(out=ek[:, 6:12, :], in0=ek[:, 6:12, :],
                                     in1=_bcast_mid(rb, 6))
                # PSUM group start/stop tracking is per partition-range:
                # each col-group (pr) of a bank needs its own start and stop.
                for hd in range(H):
                    g, pr, ch = hd // 4, hd % 2, (hd // 2) % 2
                    mm = nc.tensor.matmul(
                        out=av[g][pr * 64:(pr + 1) * 64,
                                  ch * QGS:(ch + 1) * QGS],
                        lhsT=vn_tiles[kb][:, hd * 64:(hd + 1) * 64],
                        rhs=ek[:, (hd // 4) * 4 + SLOT[hd % 4], :],
                        start=(kb == 0 and hd % 4 in (0, 1)),
                        stop=(kb == NKB - 1 and hd % 4 in (2, 3)),
                        tile_position=(0, pr * 64),
                        # sim's group-protocol tracker mis-addresses
                        # partition-offset PSUM writes; data semantics
                        # (pending-zero) are tracked per-tensor and correct
                        skip_group_check=True)
                    if kb == 0:
                        if hd % 4 in (0, 1):
                            av_first[g][pr] = mm
                        else:
                            add_dep_helper(mm.ins, av_first[g][pr].ins,
                                           reason="psum grp start")
                    if kb == NKB - 1:
                        if hd % 4 in (0, 1):
                            av_last[g][pr] = mm
                        else:
                            add_dep_helper(mm.ins, av_last[g][pr].ins,
                                           reason="psum grp stop")
            # evict attention + residual -> x2 (one DMA for all 6 row-blocks)
            xqt = p_xq.tile([128, CI, QGS], F32, tag="xq", name="xqt")
            nc.sync.dma_start(
                out=xqt,
                in_=xq[:, q0:q0 + QGS].rearrange("(a p) q -> p a q", p=128))
            for g in range(3):
                for ch in range(2):
                    p = 2 * g + ch
                    nc.vector.scalar_tensor_tensor(
                        out=x2_tiles[p][:, q0:q0 + QGS],
                        in0=av[g][:, ch * QGS:(ch + 1) * QGS],
                        scalar=0.0, in1=xqt[:, p, :], op0=ALU.add, op1=ALU.add)

        def ln2_mlp_qg(qg):
            q0 = qg * QGS
            x2b = []
            for ci in range(CI):
                bt = p_x2b.tile([128, QGS], BF16, tag=f"x2b{ci}",
                                name=f"x2b{ci}")
                nc.vector.tensor_copy(out=bt,
                                      in_=x2_tiles[ci][:, q0:q0 + QGS])
                x2b.append(bt)
            sb2, mb2 = ln_stats_and_bcast(x2b, QGS)
            h2 = []
            for ci in range(CI):
                ht = p_h2.tile([128, QGS], BF16, tag=f"h2_{ci}",
                               name=f"h2_{qg}_{ci}")
                nc.gpsimd.tensor_mul(out=ht, in0=x2b[ci], in1=sb2)
                nc.vector.tensor_add(out=ht, in0=ht, in1=mb2)
                h2.append(ht)
            # fc1 + gelu: 6 psum tiles (2 banks), 4 j-tiles each
            g1_tiles = []
            for jtq in range(DFF // 512):
                ps = p_sps.tile([128, 4, QGS], F32, tag="sp",
                                name=f"f1ps{qg}_{jtq}")
                # one accumulation group per bank (2 slots each)
                firsts, lasts = {}, {}
                for sl in range(4):
                    jt = jtq * 4 + sl
                    for ci in range(CI):
                        mm = nc.tensor.matmul(
                            out=ps[:, sl, :],
                            lhsT=wf1_tiles[ci][:, jt * 128:(jt + 1) * 128],
                            rhs=h2[ci],
                            start=(sl % 2 == 0 and ci == 0),
                            stop=(sl % 2 == 1 and ci == CI - 1),
                            skip_group_check=True)
                        if ci == 0:
                            firsts[sl] = mm
                        if ci == CI - 1:
                            lasts[sl] = mm
                for a, b in ((0, 1), (2, 3)):
                    add_dep_helper(firsts[b].ins, firsts[a].ins, reason="grp")
                    add_dep_helper(lasts[b].ins, lasts[a].ins, reason="grp")
                g1 = p_g1.tile([128, 4, QGS], BF16, tag=f"g1_{jtq}",
                               name=f"g1_{qg}_{jtq}")
                if has_c1:
                    for sl in range(4):
                        jt = jtq * 4 + sl
                        nc.scalar.activation(
                            out=g1[:, sl, :], in_=ps[:, sl, :],
                            func=AF.Gelu, bias=c1_sb[:, jt:jt + 1])
                else:
                    nc.scalar.activation(
                        out=g1.rearrange("p a b -> p (a b)"),
                        in_=ps.rearrange("p a b -> p (a b)"),
                        func=AF.Gelu)
                g1_tiles.append(g1)
            # fc2: single pass over wfc2; 6 out-tiles packed into 2 "sp"
            # psum tiles, one accumulation group per bank (2 slots each)
            psA = p_sps.tile([128, 4 * QGS], F32, tag="sp", name=f"f2A{qg}")
            psB = p_sps.tile([128, 4 * QGS], F32, tag="sp", name=f"f2B{qg}")
            slot_of = [(psA, 0), (psA, 1), (psA, 2), (psA, 3),
                       (psB, 0), (psB, 1)]
            firsts2, lasts2 = {}, {}
            for chunk in range(8):   # 3 k-tiles per DMA
                wt = p_wf2.tile([128, 3, D], BF16, tag="wf2", name="wf2t")
                nc.sync.dma_start(
                    out=wt,
                    in_=wfc2_d[chunk * 384:(chunk + 1) * 384, :]
                    .rearrange("(a p) d -> p a d", p=128))
                for jt2 in range(6):
                    pst, sl = slot_of[jt2]
                    for i in range(3):
                        ci2 = chunk * 3 + i
                        jtq, slq = ci2 // 4, ci2 % 4
                        mm = nc.tensor.matmul(
                            out=pst[:, sl * QGS:(sl + 1) * QGS],
                            lhsT=wt[:, i, jt2 * 128:(jt2 + 1) * 128],
                            rhs=g1_tiles[jtq][:, slq, :],
                            start=(chunk == 0 and i == 0 and sl % 2 == 0),
                            stop=(chunk == 7 and i == 2 and
                                  (sl % 2 == 1 or jt2 == 5)),
                            skip_group_check=True)
                        if chunk == 0 and i == 0:
                            firsts2[jt2] = mm
                        if chunk == 7 and i == 2:
                            lasts2[jt2] = mm
            for a, b in ((0, 1), (2, 3), (4, 5)):
                add_dep_helper(firsts2[b].ins, firsts2[a].ins, reason="grp")
                add_dep_helper(lasts2[b].ins, lasts2[a].ins, reason="grp")
            for jt2 in range(6):
                pst, sl = slot_of[jt2]
                yt = p_y.tile([128, QGS], F32, tag="y", name="yt")
                if has_c2:
                    nc.vector.scalar_tensor_tensor(
                        out=yt, in0=pst[:, sl * QGS:(sl + 1) * QGS],
                        scalar=c2_sb[:, jt2:jt2 + 1],
                        in1=x2_tiles[jt2][:, q0:q0 + QGS],
                        op0=ALU.add, op1=ALU.add)
                else:
                    nc.vector.scalar_tensor_tensor(
                        out=yt, in0=pst[:, sl * QGS:(sl + 1) * QGS],
                        scalar=0.0,
                        in1=x2_tiles[jt2][:, q0:q0 + QGS],
                        op0=ALU.add, op1=ALU.add)
                nc.gpsimd.dma_start(
                    out=out_t[jt2 * 128:(jt2 + 1) * 128, q0:q0 + QGS],
                    in_=yt)

        for qg in range(NQG):
            attention_qg(qg)
            ln2_mlp_qg(qg)


def _get_nc(has_cqkv, has_c1, has_c2):
    key = (has_cqkv, has_c1, has_c2)
    if key not in _CACHE:
        _CACHE[key] = build_nc(*key)
    return _CACHE[key]


def _prep_host(x, ln1_w, ln1_b, w_qkv, b_qkv, ln2_w, ln2_b, w_fc1, b_fc1,
               w_fc2, b_fc2):
    f32 = np.float32
    bf = ml_dtypes.bfloat16
    x = np.asarray(x, f32)
    ln1_w = np.asarray(ln1_w, f32); ln1_b = np.asarray(ln1_b, f32)
    ln2_w = np.asarray(ln2_w, f32); ln2_b = np.asarray(ln2_b, f32)
    w_qkv = np.asarray(w_qkv, f32); b_qkv = np.asarray(b_qkv, f32)
    w_fc1 = np.asarray(w_fc1, f32); b_fc1 = np.asarray(b_fc1, f32)
    w_fc2 = np.asarray(w_fc2, f32); b_fc2 = np.asarray(b_fc2, f32)

    scale = HD ** -0.5
    wq_eff = ln1_w[:, None] * w_qkv
    cqkv = ln1_b @ w_qkv + b_qkv
    wq_eff[:, :D] *= scale
    cqkv[:D] *= scale
    w1_eff = ln2_w[:, None] * w_fc1
    c1 = ln2_b @ w_fc1 + b_fc1
    c2 = b_fc2

    shared = {
        "w_qkv": np.ascontiguousarray(wq_eff.astype(bf)),
        "w_fc1": np.ascontiguousarray(w1_eff.astype(bf)),
        "w_fc2": np.ascontiguousarray(w_fc2.astype(bf)),
        "c_qkv": np.ascontiguousarray(cqkv.reshape(-1, 1)),
        "c_fc1": np.ascontiguousarray(c1.reshape(-1, 1)),
        "c_fc2": np.ascontiguousarray(c2.reshape(-1, 1)),
    }
    flags = (bool(np.any(cqkv)), bool(np.any(c1)), bool(np.any(c2)))

    in_maps = []
    for c in range(NCORES):
        b, qh = c // 2, c % 2
        xb = x[b]
        if qh:
            xb = np.concatenate([xb[TQ:], xb[:TQ]], axis=0)
        m = dict(shared)
        m["xt_bf"] = np.ascontiguousarray(xb.T.astype(bf))
        m["xq_f32"] = np.ascontiguousarray(xb[:TQ].T)
        in_maps.append(m)
    return in_maps, flags


def kernel(**inputs):
    global _LAST_RESULTS
    in_maps, flags = _prep_host(**inputs)
    nc = _get_nc(*flags)
    trace = bool(os.environ.get("KERNEL_TRACE"))
    try:
        res = run_bass_kernel_spmd(nc, in_maps, core_ids=list(range(NCORES)),
                                   trace=trace)
    except ModuleNotFoundError:
        # NTFF profiling hook unavailable in this axon client
        res = run_bass_kernel_spmd(nc, in_maps, core_ids=list(range(NCORES)),
                                   trace=False)
    _LAST_RESULTS = res
    y = np.empty((B, P, D), np.float32)
    for c in range(NCORES):
        b, qh = c // 2, c % 2
        y[b, qh * TQ:(qh + 1) * TQ, :] = res.results[c]["out_t"].T
    return y
